# revision 1
# baseline (speedup 1.0000x reference)
"""Complex-LSTM TRN2 kernel (8 NeuronCores, tensor-parallel over hidden units).

kernel(**inputs) takes the FULL unsharded inputs (as in setup_inputs()) and
returns the full [16, 2000, 512, 2] float32 output.

Design: 8-way tensor parallel over the 512 hidden units (64/core). Per step:
  z_slice[16,512] = sum_d hT_tile[d] @ Wrec_chunk[d] + zx_t (identity matmul)
  gates/cell update on ScalarE+VectorE in batch-major layout,
  h[16,128] -> DVE 32x32 transposes -> hT[128,16] -> remote_dma_broadcast to
  all 8 cores (XOR-relative dests; one-time Switch picks the sender slot).
The input GEMM zx = x@W + b streams on TensorE between recurrence matmuls.
"""
import sys
sys.path.insert(0, '/opt/trn_rl_repo')
import numpy as np
from concourse import bass, bacc, mybir
from concourse import bass_utils

F32 = mybir.dt.float32
AF = mybir.ActivationFunctionType
N_CORES = 8
T_STEPS = 2000

_cache = {}


def _build(T):
    ITERS = T // 8
    NBLK = ITERS + 1
    nc = bacc.Bacc("TRN2", target_bir_lowering=False, debug=False,
                   num_devices=N_CORES, detect_race_conditions=False)

    wrec = nc.dram_tensor("wrec", [1024, 512], F32, kind="ExternalInput")
    wg = nc.dram_tensor("wg", [1024, 512], F32, kind="ExternalInput")
    biasd = nc.dram_tensor("biasd", [1, 512], F32, kind="ExternalInput")
    xt = nc.dram_tensor("xt", [1024, NBLK * 128], F32, kind="ExternalInput")
    i16d = nc.dram_tensor("i16d", [128, 128], F32, kind="ExternalInput")
    onesd = nc.dram_tensor("onesd", [1, 128], F32, kind="ExternalInput")
    hout = nc.dram_tensor("hout", [T * 16, 128], F32, kind="ExternalOutput")

    wrec_sb = nc.alloc_sbuf_tensor("wrec_sb", [128, 8 * 512], F32)
    wg_sb = nc.alloc_sbuf_tensor("wg_sb", [128, 8 * 512], F32)
    bias_sb = nc.alloc_sbuf_tensor("bias_sb", [1, 512], F32)
    i16_sb = nc.alloc_sbuf_tensor("i16_sb", [128, 128], F32)
    ones_sb = nc.alloc_sbuf_tensor("ones_sb", [1, 128], F32)
    xstage = nc.alloc_sbuf_tensor("xstage", [128, 8 * 128], F32)
    ring = nc.alloc_sbuf_tensor("ring", [128, 512], F32)
    recv = [nc.alloc_sbuf_tensor(f"recv{p}", [128, 128], F32) for p in range(2)]
    src = [nc.alloc_sbuf_tensor(f"srcb{p}", [128, 32], F32) for p in range(2)]
    G = [nc.alloc_sbuf_tensor(f"G{p}", [16, 512], F32) for p in range(2)]
    TC = [nc.alloc_sbuf_tensor(f"TC{p}", [16, 128], F32) for p in range(2)]
    hb = [nc.alloc_sbuf_tensor(f"hb{p}", [32, 128], F32) for p in range(2)]
    C = nc.alloc_sbuf_tensor("C", [16, 128], F32)
    M1 = nc.alloc_sbuf_tensor("M1", [16, 128], F32)
    M2 = nc.alloc_sbuf_tensor("M2", [16, 128], F32)
    M3 = nc.alloc_sbuf_tensor("M3", [16, 128], F32)
    M4 = nc.alloc_sbuf_tensor("M4", [16, 128], F32)
    T1 = nc.alloc_sbuf_tensor("T1", [16, 128], F32)
    T2 = nc.alloc_sbuf_tensor("T2", [16, 128], F32)

    z_ps = [nc.alloc_psum_tensor(f"z_ps{p}", [16, 512], F32) for p in range(2)]
    g_ps = nc.alloc_psum_tensor("g_ps", [128, 512], F32)

    pe, act, dve, gp, sp = nc.tensor, nc.scalar, nc.vector, nc.gpsimd, nc.sync

    with (
        nc.semaphore("init_sem") as init_sem,
        nc.semaphore("prep_sem") as prep_sem,
        nc.semaphore("lsem") as lsem,
        nc.semaphore("rsem") as rsem,
        nc.semaphore("pe_z") as pe_z,
        nc.semaphore("pe_gemm") as pe_gemm,
        nc.semaphore("act1") as act1,
        nc.semaphore("act2") as act2,
        nc.semaphore("act_zx") as act_zx,
        nc.semaphore("dve_c") as dve_c,
        nc.semaphore("dve_h") as dve_h,
        nc.semaphore("dve_cp") as dve_cp,
        nc.semaphore("x_sem") as x_sem,
        nc.semaphore("out_dma") as out_dma,
    ):
        n_init = 0
        for d in range(8):
            sp.dma_start(out=wrec_sb[:, 512 * d:512 * d + 512],
                         in_=wrec[128 * d:128 * d + 128, :]).then_inc(init_sem, 16)
            sp.dma_start(out=wg_sb[:, 512 * d:512 * d + 512],
                         in_=wg[128 * d:128 * d + 128, :]).then_inc(init_sem, 16)
            n_init += 2
        for dst, srct in ((bias_sb, biasd), (i16_sb, i16d), (ones_sb, onesd)):
            sp.dma_start(out=dst[:], in_=srct[:]).then_inc(init_sem, 16)
            n_init += 1
        sp.wait_ge(init_sem, 16 * n_init)
        dve.memset(recv[0][:], 0.0)
        dve.memset(C[:], 0.0)
        dve.memset(hb[0][:], 0.0)
        dve.memset(hb[1][:], 0.0)
        dve.memset(M1[0:1, 0:1], 0.0).then_inc(dve_cp, 2)
        dve.memset(M2[0:1, 0:1], 0.0).then_inc(dve_c, 2)
        dve.memset(M4[0:1, 0:1], 0.0).then_inc(out_dma, 32)
        for k in range(2):
            gp.remote_sem_update_broadcast(
                remote_sem=rsem, local_sem=lsem,
                rdests=[(0, kk) for kk in range(8)],
            ).then_inc(prep_sem)
            gp.wait_ge(prep_sem, k + 1)
            gp.trigger_dma(1)
        nc.all_engine_barrier()

        xcol = sp.alloc_register("xcol"); sp.reg_mov(xcol, 0)
        orow = sp.alloc_register("orow"); sp.reg_mov(orow, 0)
        r_sp_dveh = sp.alloc_register("r_sp_dveh"); sp.reg_mov(r_sp_dveh, 0)
        r_sp_gemm = sp.alloc_register("r_sp_gemm"); sp.reg_mov(r_sp_gemm, 0)

        def x_block_dma():
            xoff = sp.snap(xcol)
            for k in range(8):
                sp.dma_start(
                    out=xstage[:, 128 * k:128 * k + 128],
                    in_=xt[128 * k:128 * k + 128, bass.ds(xoff, 128)],
                ).then_inc(x_sem, 16)
            sp.reg_add(xcol, xcol, 128)

        x_block_dma()

        pe.wait_ge(x_sem, 128)
        for k in range(8):
            pe.matmul(g_ps[:], xstage[:, 128 * k:128 * k + 128],
                      wg_sb[:, 512 * k:512 * k + 512],
                      start=(k == 0), stop=False, skip_group_check=True)
        pe.matmul(g_ps[:], ones_sb[0:1, :], bias_sb[0:1, :],
                  start=False, stop=True, skip_group_check=True).then_inc(pe_gemm)

        act.wait_ge(pe_gemm, 1)
        act.activation(ring[:], g_ps[:], AF.Copy).then_inc(act_zx)

        r_pe_rsem = pe.alloc_register("r_pe_rsem"); pe.reg_mov(r_pe_rsem, 32)
        r_pe_x = pe.alloc_register("r_pe_x"); pe.reg_mov(r_pe_x, 128)
        r_pe_zx = pe.alloc_register("r_pe_zx"); pe.reg_mov(r_pe_zx, 0)
        r_a_pez = act.alloc_register("r_a_pez"); act.reg_mov(r_a_pez, 0)
        r_a_war = act.alloc_register("r_a_war"); act.reg_mov(r_a_war, 0)
        r_a_dvec = act.alloc_register("r_a_dvec"); act.reg_mov(r_a_dvec, 2)
        r_a_gemm = act.alloc_register("r_a_gemm"); act.reg_mov(r_a_gemm, 1)
        r_v_act1 = dve.alloc_register("r_v_act1"); dve.reg_mov(r_v_act1, 0)
        r_v_act2 = dve.alloc_register("r_v_act2"); dve.reg_mov(r_v_act2, 0)
        r_v_odma = dve.alloc_register("r_v_odma"); dve.reg_mov(r_v_odma, 0)
        r_g_prep = gp.alloc_register("r_g_prep"); gp.reg_mov(r_g_prep, 2)
        r_g_cp = gp.alloc_register("r_g_cp"); gp.reg_mov(r_g_cp, 2)

        with pe.Fori(0, ITERS):
            pe.reg_add(r_pe_zx, r_pe_zx, 1)
            for j in range(8):
                par = (j + 1) % 2
                rpar = j % 2
                pe.wait_ge(rsem, r_pe_rsem)
                pe.reg_add(r_pe_rsem, r_pe_rsem, 16)
                for d in range(8):
                    pe.matmul(z_ps[par][:],
                              recv[rpar][:, 16 * d:16 * d + 16],
                              wrec_sb[:, 512 * d:512 * d + 512],
                              start=(d == 0), stop=False, skip_group_check=True)
                if j == 0:
                    pe.wait_ge(act_zx, r_pe_zx)
                pe.matmul(z_ps[par][:], i16_sb[:, 16 * j:16 * j + 16],
                          ring[:],
                          start=False, stop=True,
                          skip_group_check=True).then_inc(pe_z)
                if j >= 1:
                    if j == 1:
                        pe.reg_add(r_pe_x, r_pe_x, 128)
                        pe.wait_ge(x_sem, r_pe_x)
                    k = j - 1
                    pe.matmul(g_ps[:], xstage[:, 128 * k:128 * k + 128],
                              wg_sb[:, 512 * k:512 * k + 512],
                              start=(k == 0), stop=False, skip_group_check=True)
                if j == 7:
                    pe.matmul(g_ps[:], xstage[:, 128 * 7:128 * 7 + 128],
                              wg_sb[:, 512 * 7:512 * 7 + 512],
                              start=False, stop=False, skip_group_check=True)
                    pe.matmul(g_ps[:], ones_sb[0:1, :], bias_sb[0:1, :],
                              start=False, stop=True,
                              skip_group_check=True).then_inc(pe_gemm)

        with act.Fori(0, ITERS):
            for j in range(8):
                par = (j + 1) % 2
                act.reg_add(r_a_pez, r_a_pez, 1)
                act.wait_ge(pe_z, r_a_pez)
                act.reg_add(r_a_war, r_a_war, 1)
                act.wait_ge(dve_c, r_a_war)
                act.activation(G[par][:, 0:128], z_ps[par][:, 0:128], AF.Tanh)
                act.activation(G[par][:, 128:512], z_ps[par][:, 128:512],
                               AF.Sigmoid).then_inc(act1)
                act.reg_add(r_a_dvec, r_a_dvec, 1)
                act.wait_ge(dve_c, r_a_dvec)
                act.activation(TC[par][:], C[:], AF.Tanh).then_inc(act2)
            act.reg_add(r_a_gemm, r_a_gemm, 1)
            act.wait_ge(pe_gemm, r_a_gemm)
            act.activation(ring[:], g_ps[:], AF.Copy).then_inc(act_zx)

        with dve.Fori(0, ITERS):
            for j in range(8):
                par = (j + 1) % 2
                dve.reg_add(r_v_act1, r_v_act1, 1)
                dve.wait_ge(act1, r_v_act1)
                Gp = G[par]
                dve.tensor_mul(M1[:], Gp[:, 0:128], Gp[:, 128:256])
                dve.tensor_mul(M2[:, 0:64], Gp[:, 0:64], Gp[:, 192:256])
                dve.tensor_mul(M2[:, 64:128], Gp[:, 64:128], Gp[:, 128:192])
                dve.tensor_mul(M3[:], Gp[:, 256:384], C[:])
                dve.tensor_mul(M4[:, 0:64], Gp[:, 256:320], C[:, 64:128])
                dve.tensor_mul(M4[:, 64:128], Gp[:, 320:384], C[:, 0:64])
                dve.drain()
                dve.tensor_add(T1[:], M1[:], M3[:])
                dve.tensor_add(T2[:], M2[:], M4[:])
                dve.drain()
                dve.tensor_sub(C[:, 0:64], T1[:, 0:64], T1[:, 64:128])
                dve.tensor_add(C[:, 64:128], T2[:, 0:64],
                               T2[:, 64:128]).then_inc(dve_c)
                dve.reg_add(r_v_act2, r_v_act2, 1)
                dve.wait_ge(act2, r_v_act2)
                dve.tensor_mul(M1[:], Gp[:, 384:512], TC[par][:])
                dve.tensor_mul(M2[:, 0:64], Gp[:, 384:448], TC[par][:, 64:128])
                dve.tensor_mul(M2[:, 64:128], Gp[:, 448:512], TC[par][:, 0:64])
                dve.reg_add(r_v_odma, r_v_odma, 16)
                dve.wait_ge(out_dma, r_v_odma)
                dve.drain()
                dve.tensor_sub(hb[par][0:16, 0:64], M1[:, 0:64], M1[:, 64:128])
                dve.tensor_add(hb[par][0:16, 64:128], M2[:, 0:64],
                               M2[:, 64:128]).then_inc(dve_h)
                dve.drain()
                for kk in range(4):
                    inst = dve.transpose(
                        src[par][32 * kk:32 * kk + 32, 0:32],
                        hb[par][0:32, 32 * kk:32 * kk + 32])
                inst.then_inc(dve_cp)

        with sp.Fori(0, ITERS):
            sp.reg_add(r_sp_gemm, r_sp_gemm, 1)
            sp.wait_ge(pe_gemm, r_sp_gemm)
            x_block_dma()
            for j in range(8):
                par = (j + 1) % 2
                sp.reg_add(r_sp_dveh, r_sp_dveh, 1)
                sp.wait_ge(dve_h, r_sp_dveh)
                sp.dma_start(out=hout[bass.ds(sp.snap(orow), 16), :],
                             in_=hb[par][0:16, :]).then_inc(out_dma, 16)
                sp.reg_add(orow, orow, 16)

        pid = gp.partition_id()
        for c in gp.Switch(pid, 8):
            gp.remote_dma_broadcast(
                recv[1][:, 16 * c:16 * c + 16], src[1][:, 0:16],
                remote_sem=rsem, local_sem=lsem,
                rdests=[(0, k) for k in range(8)],
            ).then_inc(prep_sem)
            with gp.Fori(0, T // 2):
                for tp in (1, 0):
                    gp.reg_add(r_g_prep, r_g_prep, 1)
                    gp.wait_ge(prep_sem, r_g_prep)
                    gp.reg_add(r_g_cp, r_g_cp, 1)
                    gp.wait_ge(dve_cp, r_g_cp)
                    gp.trigger_dma(1)
                    gp.remote_dma_broadcast(
                        recv[1 - tp][:, 16 * c:16 * c + 16],
                        src[1 - tp][:, 0:16],
                        remote_sem=rsem, local_sem=lsem,
                        rdests=[(0, k) for k in range(8)],
                    ).then_inc(prep_sem)
            gp.reg_add(r_g_prep, r_g_prep, 1)
            gp.wait_ge(prep_sem, r_g_prep)
            gp.trigger_dma(1)

        gp.wait_ge(rsem, 32 + 16 * (T + 1))
        gp.wait_ge(lsem, 32 + 16 * (T + 1))
        sp.wait_ge(out_dma, 32 + 16 * T)
        nc.all_engine_barrier()

    nc.compile()
    return nc


def _prep_inputs(x_real, x_imag, W_real, W_imag, U_real, U_imag,
                 b_real, b_imag, T):
    ITERS = T // 8
    NBLK = ITERS + 1
    xt = np.zeros((1024, NBLK * 128), np.float32)
    xr = np.asarray(x_real[:, :T], np.float32)
    xi = np.asarray(x_imag[:, :T], np.float32)
    xt[0:512, :T * 16] = xr.transpose(2, 1, 0).reshape(512, T * 16)
    xt[512:1024, :T * 16] = xi.transpose(2, 1, 0).reshape(512, T * 16)

    i16 = np.eye(128, dtype=np.float32)
    ones1 = np.ones((1, 128), np.float32)
    Wr = np.asarray(W_real, np.float32); Wi = np.asarray(W_imag, np.float32)
    Ur = np.asarray(U_real, np.float32); Ui = np.asarray(U_imag, np.float32)
    br = np.asarray(b_real, np.float32); bi = np.asarray(b_imag, np.float32)

    in_maps = []
    for c in range(N_CORES):
        cols = np.zeros(512, np.int64)
        part = np.zeros(512, np.int64)
        for g in range(4):
            for pt in range(2):
                base = 128 * g + 64 * pt
                cols[base:base + 64] = 512 * g + 64 * c + np.arange(64)
                part[base:base + 64] = pt
        re = part == 0

        wgc = np.zeros((1024, 512), np.float32)
        wgc[0:512, re] = Wr[:, cols[re]]
        wgc[0:512, ~re] = Wi[:, cols[~re]]
        wgc[512:1024, re] = -Wi[:, cols[re]]
        wgc[512:1024, ~re] = Wr[:, cols[~re]]

        wrc = np.zeros((1024, 512), np.float32)
        for d in range(8):
            hf = 64 * d + np.arange(64)
            r0 = 128 * d
            wrc[r0:r0 + 64, re] = Ur[hf][:, cols[re]]
            wrc[r0:r0 + 64, ~re] = Ui[hf][:, cols[~re]]
            wrc[r0 + 64:r0 + 128, re] = -Ui[hf][:, cols[re]]
            wrc[r0 + 64:r0 + 128, ~re] = Ur[hf][:, cols[~re]]

        biasc = np.zeros((1, 512), np.float32)
        biasc[0, re] = br[cols[re]]
        biasc[0, ~re] = bi[cols[~re]]

        in_maps.append({
            "wrec": wrc, "wg": wgc, "biasd": biasc, "xt": xt,
            "i16d": i16, "onesd": ones1,
        })
    return in_maps


def kernel(x_real, x_imag, W_real, W_imag, U_real, U_imag, b_real, b_imag):
    T = int(np.asarray(x_real).shape[1])
    if T not in _cache:
        _cache[T] = _build(T)
    nc = _cache[T]
    in_maps = _prep_inputs(x_real, x_imag, W_real, W_imag, U_real, U_imag,
                           b_real, b_imag, T)
    res = bass_utils.run_bass_kernel_spmd(nc, in_maps,
                                          core_ids=list(range(N_CORES)))
    out = np.zeros((16, T, 512, 2), np.float32)
    for c in range(N_CORES):
        h = res.results[c]["hout"].reshape(T, 16, 128)
        out[:, :, 64 * c:64 * c + 64, 0] = h[:, :, 0:64].transpose(1, 0, 2)
        out[:, :, 64 * c:64 * c + 64, 1] = h[:, :, 64:128].transpose(1, 0, 2)
    return out



# revision 11
# speedup vs baseline: 1.5558x; 1.5558x over previous
"""Complex-LSTM TRN2 kernel (8 NeuronCores, tensor-parallel over hidden units).

kernel(**inputs) takes the FULL unsharded inputs (as in setup_inputs()) and
returns the full [16, 2000, 512, 2] float32 output.

Design: 8-way tensor parallel over the 512 hidden units (64/core). Per step:
  z_slice[16,512] = sum_d hT_tile[d] @ Wrec_chunk[d] + zx_t (identity matmul)
  gates/cell update on ScalarE+VectorE in batch-major layout,
  h[16,128] -> DVE 32x32 transposes -> hT[128,16] -> remote_dma_broadcast to
  all 8 cores (XOR-relative dests; one-time Switch picks the sender slot).
The input GEMM zx = x@W + b streams on TensorE between recurrence matmuls.
"""
import sys
sys.path.insert(0, '/opt/trn_rl_repo')
import numpy as np
from concourse import bass, bacc, mybir
from concourse import bass_utils

F32 = mybir.dt.float32
F32R = mybir.dt.float32r
BF16 = mybir.dt.bfloat16
AF = mybir.ActivationFunctionType
N_CORES = 8
T_STEPS = 2000

_cache = {}


def _build(T):
    ITERS = T // 8
    NBLK = ITERS + 1
    nc = bacc.Bacc("TRN2", target_bir_lowering=False, debug=False,
                   num_devices=N_CORES, detect_race_conditions=False)

    wrec = nc.dram_tensor("wrec", [1024, 512], BF16, kind="ExternalInput")
    wg = nc.dram_tensor("wg", [1024, 512], F32R, kind="ExternalInput")
    biasd = nc.dram_tensor("biasd", [1, 512], F32R, kind="ExternalInput")
    xt = nc.dram_tensor("xt", [1024, NBLK * 128], F32R, kind="ExternalInput")
    i16d = nc.dram_tensor("i16d", [128, 128], BF16, kind="ExternalInput")
    onesd = nc.dram_tensor("onesd", [1, 128], F32R, kind="ExternalInput")
    hout = nc.dram_tensor("hout", [T * 16, 128], BF16, kind="ExternalOutput")

    wrec_sb = nc.alloc_sbuf_tensor("wrec_sb", [128, 8 * 512], BF16)
    wg_sb = nc.alloc_sbuf_tensor("wg_sb", [128, 8 * 512], F32R)
    bias_sb = nc.alloc_sbuf_tensor("bias_sb", [1, 512], F32R)
    i16_sb = nc.alloc_sbuf_tensor("i16_sb", [128, 128], BF16)
    ones_sb = nc.alloc_sbuf_tensor("ones_sb", [1, 128], F32R)
    xstage = nc.alloc_sbuf_tensor("xstage", [128, 8 * 128], F32R)
    ring = nc.alloc_sbuf_tensor("ring", [128, 512], BF16)
    recv = [nc.alloc_sbuf_tensor(f"recv{p}", [128, 128], BF16) for p in range(2)]
    src = [nc.alloc_sbuf_tensor(f"srcb{p}", [128, 32], BF16) for p in range(2)]
    G = [nc.alloc_sbuf_tensor(f"G{p}", [16, 512], F32) for p in range(2)]
    TC = [nc.alloc_sbuf_tensor(f"TC{p}", [16, 128], F32) for p in range(2)]
    hb = [nc.alloc_sbuf_tensor(f"hb{p}", [32, 128], BF16) for p in range(2)]
    C = nc.alloc_sbuf_tensor("C", [16, 128], F32)
    M1 = nc.alloc_sbuf_tensor("M1", [16, 128], F32)
    M2 = nc.alloc_sbuf_tensor("M2", [16, 128], F32)
    M3 = nc.alloc_sbuf_tensor("M3", [16, 128], F32)
    M4 = nc.alloc_sbuf_tensor("M4", [16, 128], F32)
    T1 = nc.alloc_sbuf_tensor("T1", [16, 128], F32)
    T2 = nc.alloc_sbuf_tensor("T2", [16, 128], F32)

    z_ps = [nc.alloc_psum_tensor(f"z_ps{p}", [16, 512], F32) for p in range(2)]
    g_ps = nc.alloc_psum_tensor("g_ps", [128, 512], F32)

    pe, act, dve, gp, sp = nc.tensor, nc.scalar, nc.vector, nc.gpsimd, nc.sync

    with (
        nc.semaphore("init_sem") as init_sem,
        nc.semaphore("prep_sem") as prep_sem,
        nc.semaphore("lsem") as lsem,
        nc.semaphore("rsem") as rsem,
        nc.semaphore("pe_z") as pe_z,
        nc.semaphore("pe_gemm") as pe_gemm,
        nc.semaphore("act1") as act1,
        nc.semaphore("act2") as act2,
        nc.semaphore("act_zx") as act_zx,
        nc.semaphore("dve_c") as dve_c,
        nc.semaphore("dve_h") as dve_h,
        nc.semaphore("dve_cp") as dve_cp,
        nc.semaphore("x_sem") as x_sem,
        nc.semaphore("out_dma") as out_dma,
    ):
        n_init = 0
        for d in range(8):
            sp.dma_start(out=wrec_sb[:, 512 * d:512 * d + 512],
                         in_=wrec[128 * d:128 * d + 128, :]).then_inc(init_sem, 16)
            sp.dma_start(out=wg_sb[:, 512 * d:512 * d + 512],
                         in_=wg[128 * d:128 * d + 128, :]).then_inc(init_sem, 16)
            n_init += 2
        for dst, srct in ((bias_sb, biasd), (i16_sb, i16d), (ones_sb, onesd)):
            sp.dma_start(out=dst[:], in_=srct[:]).then_inc(init_sem, 16)
            n_init += 1
        sp.wait_ge(init_sem, 16 * n_init)
        # dummy sigmoid pins the ACT func table to sigmoid_and_others (which
        # also holds tanh+copy) so the loop body never reloads tables
        act.wait_ge(init_sem, 16 * n_init)
        act.activation(T1[0:1, 0:1], i16_sb[0:1, 0:1], AF.Sigmoid)
        dve.memset(recv[0][:], 0.0)
        dve.memset(C[:], 0.0)
        dve.memset(hb[0][:], 0.0)
        dve.memset(hb[1][:], 0.0)
        dve.memset(M1[0:1, 0:1], 0.0).then_inc(dve_cp, 2)
        dve.memset(M2[0:1, 0:1], 0.0).then_inc(dve_c, 2)
        dve.memset(M4[0:1, 0:1], 0.0).then_inc(out_dma, 32)
        for k in range(2):
            gp.remote_sem_update_broadcast(
                remote_sem=rsem, local_sem=lsem,
                rdests=[(0, kk) for kk in range(8)],
            ).then_inc(prep_sem)
            gp.wait_ge(prep_sem, k + 1)
            gp.trigger_dma(1)
        nc.all_engine_barrier()

        xcol = sp.alloc_register("xcol"); sp.reg_mov(xcol, 0)
        orow = sp.alloc_register("orow"); sp.reg_mov(orow, 0)
        r_sp_dveh = sp.alloc_register("r_sp_dveh"); sp.reg_mov(r_sp_dveh, 0)
        r_sp_gemm = sp.alloc_register("r_sp_gemm"); sp.reg_mov(r_sp_gemm, 0)

        def x_block_dma():
            xoff = sp.snap(xcol)
            for k in range(8):
                sp.dma_start(
                    out=xstage[:, 128 * k:128 * k + 128],
                    in_=xt[128 * k:128 * k + 128, bass.ds(xoff, 128)],
                ).then_inc(x_sem, 16)
            sp.reg_add(xcol, xcol, 128)

        x_block_dma()

        pe.wait_ge(x_sem, 128)
        for k in range(8):
            pe.matmul(g_ps[:], xstage[:, 128 * k:128 * k + 128],
                      wg_sb[:, 512 * k:512 * k + 512],
                      start=(k == 0), stop=False, skip_group_check=True)
        pe.matmul(g_ps[:], ones_sb[0:1, :],
                  bias_sb[0:1, :],
                  start=False, stop=True, skip_group_check=True).then_inc(pe_gemm)

        act.wait_ge(pe_gemm, 1)
        act.activation(ring[:], g_ps[:], AF.Copy).then_inc(act_zx)

        r_pe_rsem = pe.alloc_register("r_pe_rsem"); pe.reg_mov(r_pe_rsem, 32)
        r_pe_x = pe.alloc_register("r_pe_x"); pe.reg_mov(r_pe_x, 128)
        r_pe_zx = pe.alloc_register("r_pe_zx"); pe.reg_mov(r_pe_zx, 0)
        r_a_pez = act.alloc_register("r_a_pez"); act.reg_mov(r_a_pez, 0)
        r_a_war = act.alloc_register("r_a_war"); act.reg_mov(r_a_war, 0)
        r_a_dvec = act.alloc_register("r_a_dvec"); act.reg_mov(r_a_dvec, 2)
        r_a_gemm = act.alloc_register("r_a_gemm"); act.reg_mov(r_a_gemm, 1)
        r_v_act1 = dve.alloc_register("r_v_act1"); dve.reg_mov(r_v_act1, 0)
        r_v_act2 = dve.alloc_register("r_v_act2"); dve.reg_mov(r_v_act2, 0)
        r_v_odma = dve.alloc_register("r_v_odma"); dve.reg_mov(r_v_odma, 0)
        r_g_prep = gp.alloc_register("r_g_prep"); gp.reg_mov(r_g_prep, 2)
        r_g_cp = gp.alloc_register("r_g_cp"); gp.reg_mov(r_g_cp, 2)

        with pe.Fori(0, ITERS):
            pe.reg_add(r_pe_zx, r_pe_zx, 1)
            for j in range(8):
                par = (j + 1) % 2
                rpar = j % 2
                pe.wait_ge(rsem, r_pe_rsem)
                pe.reg_add(r_pe_rsem, r_pe_rsem, 16)
                for d in range(8):
                    pe.matmul(z_ps[par][:],
                              recv[rpar][:, 16 * d:16 * d + 16],
                              wrec_sb[:, 512 * d:512 * d + 512],
                              start=(d == 0), stop=False, skip_group_check=True)
                if j == 0:
                    pe.wait_ge(act_zx, r_pe_zx)
                pe.matmul(z_ps[par][:], i16_sb[:, 16 * j:16 * j + 16],
                          ring[:],
                          start=False, stop=True,
                          skip_group_check=True).then_inc(pe_z)
                if j >= 1:
                    if j == 1:
                        pe.reg_add(r_pe_x, r_pe_x, 128)
                        pe.wait_ge(x_sem, r_pe_x)
                    k = j - 1
                    pe.matmul(g_ps[:], xstage[:, 128 * k:128 * k + 128],
                              wg_sb[:, 512 * k:512 * k + 512],
                              start=(k == 0), stop=False, skip_group_check=True)
                if j == 7:
                    pe.matmul(g_ps[:], xstage[:, 128 * 7:128 * 7 + 128],
                              wg_sb[:, 512 * 7:512 * 7 + 512],
                              start=False, stop=False, skip_group_check=True)
                    pe.matmul(g_ps[:], ones_sb[0:1, :],
                              bias_sb[0:1, :],
                              start=False, stop=True,
                              skip_group_check=True).then_inc(pe_gemm)

        with act.Fori(0, ITERS):
            for j in range(8):
                par = (j + 1) % 2
                act.reg_add(r_a_pez, r_a_pez, 1)
                act.wait_ge(pe_z, r_a_pez)
                act.reg_add(r_a_war, r_a_war, 1)
                act.wait_ge(dve_c, r_a_war)
                act.activation(G[par][:, 0:128], z_ps[par][:, 0:128], AF.Tanh)
                act.activation(G[par][:, 128:512], z_ps[par][:, 128:512],
                               AF.Sigmoid).then_inc(act1)
                act.reg_add(r_a_dvec, r_a_dvec, 1)
                act.wait_ge(dve_c, r_a_dvec)
                act.activation(TC[par][:], C[:], AF.Tanh).then_inc(act2)
            act.reg_add(r_a_gemm, r_a_gemm, 1)
            act.wait_ge(pe_gemm, r_a_gemm)
            act.activation(ring[:], g_ps[:], AF.Copy).then_inc(act_zx)

        with dve.Fori(0, ITERS):
            for j in range(8):
                par = (j + 1) % 2
                dve.reg_add(r_v_act1, r_v_act1, 1)
                dve.wait_ge(act1, r_v_act1)
                Gp = G[par]
                dve.tensor_mul(M1[:], Gp[:, 0:128], Gp[:, 128:256])
                dve.tensor_mul(M2[:, 0:64], Gp[:, 0:64], Gp[:, 192:256])
                dve.tensor_mul(M2[:, 64:128], Gp[:, 64:128], Gp[:, 128:192])
                dve.tensor_mul(M3[:], Gp[:, 256:384], C[:])
                dve.tensor_mul(M4[:, 0:64], Gp[:, 256:320], C[:, 64:128])
                dve.tensor_mul(M4[:, 64:128], Gp[:, 320:384], C[:, 0:64])
                dve.drain()
                dve.tensor_add(T1[:], M1[:], M3[:])
                dve.tensor_add(T2[:], M2[:], M4[:])
                dve.drain()
                dve.tensor_sub(C[:, 0:64], T1[:, 0:64], T1[:, 64:128])
                dve.tensor_add(C[:, 64:128], T2[:, 0:64],
                               T2[:, 64:128]).then_inc(dve_c)
                dve.reg_add(r_v_act2, r_v_act2, 1)
                dve.wait_ge(act2, r_v_act2)
                dve.tensor_mul(M1[:], Gp[:, 384:512], TC[par][:])
                dve.tensor_mul(M2[:, 0:64], Gp[:, 384:448], TC[par][:, 64:128])
                dve.tensor_mul(M2[:, 64:128], Gp[:, 448:512], TC[par][:, 0:64])
                dve.reg_add(r_v_odma, r_v_odma, 16)
                dve.wait_ge(out_dma, r_v_odma)
                dve.drain()
                dve.tensor_sub(hb[par][0:16, 0:64], M1[:, 0:64], M1[:, 64:128])
                dve.tensor_add(hb[par][0:16, 64:128], M2[:, 0:64],
                               M2[:, 64:128]).then_inc(dve_h)
                dve.drain()
                for kk in range(4):
                    inst = dve.transpose(
                        src[par][32 * kk:32 * kk + 32, 0:32],
                        hb[par][0:32, 32 * kk:32 * kk + 32])
                inst.then_inc(dve_cp)

        with sp.Fori(0, ITERS):
            sp.reg_add(r_sp_gemm, r_sp_gemm, 1)
            sp.wait_ge(pe_gemm, r_sp_gemm)
            x_block_dma()
            for j in range(8):
                par = (j + 1) % 2
                sp.reg_add(r_sp_dveh, r_sp_dveh, 1)
                sp.wait_ge(dve_h, r_sp_dveh)
                sp.dma_start(out=hout[bass.ds(sp.snap(orow), 16), :],
                             in_=hb[par][0:16, :]).then_inc(out_dma, 16)
                sp.reg_add(orow, orow, 16)

        pid = gp.partition_id()
        for c in gp.Switch(pid, 8):
            gp.remote_dma_broadcast(
                recv[1][:, 16 * c:16 * c + 16], src[1][:, 0:16],
                remote_sem=rsem, local_sem=lsem,
                rdests=[(0, k) for k in range(8)],
            ).then_inc(prep_sem)
            with gp.Fori(0, T // 2):
                for tp in (1, 0):
                    gp.reg_add(r_g_prep, r_g_prep, 1)
                    gp.wait_ge(prep_sem, r_g_prep)
                    gp.reg_add(r_g_cp, r_g_cp, 1)
                    gp.wait_ge(dve_cp, r_g_cp)
                    gp.trigger_dma(1)
                    gp.remote_dma_broadcast(
                        recv[1 - tp][:, 16 * c:16 * c + 16],
                        src[1 - tp][:, 0:16],
                        remote_sem=rsem, local_sem=lsem,
                        rdests=[(0, k) for k in range(8)],
                    ).then_inc(prep_sem)
            gp.reg_add(r_g_prep, r_g_prep, 1)
            gp.wait_ge(prep_sem, r_g_prep)
            gp.trigger_dma(1)

        gp.wait_ge(rsem, 32 + 16 * (T + 1))
        gp.wait_ge(lsem, 32 + 16 * (T + 1))
        sp.wait_ge(out_dma, 32 + 16 * T)
        nc.all_engine_barrier()

    nc.compile()
    return nc


def _round_tf32(a):
    """Zero the low 13 mantissa bits (tf32-style rounding for f32r)."""
    b = np.ascontiguousarray(a, np.float32).view(np.uint32)
    b = (b + 0x1000) & np.uint32(0xFFFFE000)
    return b.view(np.float32)


def _prep_inputs(x_real, x_imag, W_real, W_imag, U_real, U_imag,
                 b_real, b_imag, T):
    import ml_dtypes
    BF = ml_dtypes.bfloat16
    ITERS = T // 8
    NBLK = ITERS + 1
    xt = np.zeros((1024, NBLK * 128), np.float32)
    xr = np.asarray(x_real[:, :T], np.float32)
    xi = np.asarray(x_imag[:, :T], np.float32)
    xt[0:512, :T * 16] = xr.transpose(2, 1, 0).reshape(512, T * 16)
    xt[512:1024, :T * 16] = xi.transpose(2, 1, 0).reshape(512, T * 16)
    xt = _round_tf32(xt)

    i16 = np.eye(128, dtype=BF)
    ones1 = np.ones((1, 128), np.float32)
    Wr = np.asarray(W_real, np.float32); Wi = np.asarray(W_imag, np.float32)
    Ur = np.asarray(U_real, np.float32); Ui = np.asarray(U_imag, np.float32)
    br = np.asarray(b_real, np.float32); bi = np.asarray(b_imag, np.float32)

    in_maps = []
    for c in range(N_CORES):
        cols = np.zeros(512, np.int64)
        part = np.zeros(512, np.int64)
        for g in range(4):
            for pt in range(2):
                base = 128 * g + 64 * pt
                cols[base:base + 64] = 512 * g + 64 * c + np.arange(64)
                part[base:base + 64] = pt
        re = part == 0

        wgc = np.zeros((1024, 512), np.float32)
        wgc[0:512, re] = Wr[:, cols[re]]
        wgc[0:512, ~re] = Wi[:, cols[~re]]
        wgc[512:1024, re] = -Wi[:, cols[re]]
        wgc[512:1024, ~re] = Wr[:, cols[~re]]

        wrc = np.zeros((1024, 512), np.float32)
        for d in range(8):
            hf = 64 * d + np.arange(64)
            r0 = 128 * d
            wrc[r0:r0 + 64, re] = Ur[hf][:, cols[re]]
            wrc[r0:r0 + 64, ~re] = Ui[hf][:, cols[~re]]
            wrc[r0 + 64:r0 + 128, re] = -Ui[hf][:, cols[re]]
            wrc[r0 + 64:r0 + 128, ~re] = Ur[hf][:, cols[~re]]

        biasc = np.zeros((1, 512), np.float32)
        biasc[0, re] = br[cols[re]]
        biasc[0, ~re] = bi[cols[~re]]

        in_maps.append({
            "wrec": wrc.astype(BF), "wg": _round_tf32(wgc),
            "biasd": _round_tf32(biasc), "xt": xt,
            "i16d": i16, "onesd": ones1,
        })
    return in_maps


def kernel(x_real, x_imag, W_real, W_imag, U_real, U_imag, b_real, b_imag):
    T = int(np.asarray(x_real).shape[1])
    if T not in _cache:
        _cache[T] = _build(T)
    nc = _cache[T]
    in_maps = _prep_inputs(x_real, x_imag, W_real, W_imag, U_real, U_imag,
                           b_real, b_imag, T)
    res = bass_utils.run_bass_kernel_spmd(nc, in_maps,
                                          core_ids=list(range(N_CORES)))
    out = np.zeros((16, T, 512, 2), np.float32)
    for c in range(N_CORES):
        h = np.asarray(res.results[c]["hout"], np.float32).reshape(T, 16, 128)
        out[:, :, 64 * c:64 * c + 64, 0] = h[:, :, 0:64].transpose(1, 0, 2)
        out[:, :, 64 * c:64 * c + 64, 1] = h[:, :, 64:128].transpose(1, 0, 2)
    return out



# revision 14
# speedup vs baseline: 1.7324x; 1.1135x over previous
"""Complex-LSTM TRN2 kernel (8 NeuronCores, tensor-parallel over hidden units).

kernel(**inputs) takes the FULL unsharded inputs (as in setup_inputs()) and
returns the full [16, 2000, 512, 2] float32 output.

Design: 8-way tensor parallel over the 512 hidden units (64/core). Per step:
  z_slice[16,512] = sum_d hT_tile[d] @ Wrec_chunk[d] + zx_t (identity matmul)
  gates/cell update on ScalarE+VectorE in batch-major layout,
  h[16,128] -> DVE 32x32 transposes -> hT[128,16] -> remote_dma_broadcast to
  all 8 cores (XOR-relative dests; one-time Switch picks the sender slot).
The input GEMM zx = x@W + b streams on TensorE between recurrence matmuls.
"""
import sys
sys.path.insert(0, '/opt/trn_rl_repo')
import numpy as np
from concourse import bass, bacc, mybir
from concourse import bass_utils

F32 = mybir.dt.float32
F32R = mybir.dt.float32r
BF16 = mybir.dt.bfloat16
AF = mybir.ActivationFunctionType
N_CORES = 8
T_STEPS = 2000

_cache = {}


def _build(T):
    ITERS = T // 8
    NBLK = ITERS + 1
    nc = bacc.Bacc("TRN2", target_bir_lowering=False, debug=False,
                   num_devices=N_CORES, detect_race_conditions=False)

    wrec = nc.dram_tensor("wrec", [1024, 512], BF16, kind="ExternalInput")
    wg = nc.dram_tensor("wg", [1024, 512], F32R, kind="ExternalInput")
    biasd = nc.dram_tensor("biasd", [1, 512], F32R, kind="ExternalInput")
    xt = nc.dram_tensor("xt", [1024, NBLK * 128], F32R, kind="ExternalInput")
    i16d = nc.dram_tensor("i16d", [128, 128], BF16, kind="ExternalInput")
    onesd = nc.dram_tensor("onesd", [1, 128], F32R, kind="ExternalInput")
    hout = nc.dram_tensor("hout", [T * 16, 128], BF16, kind="ExternalOutput")

    wrec_sb = nc.alloc_sbuf_tensor("wrec_sb", [128, 8 * 512], BF16)
    wg_sb = nc.alloc_sbuf_tensor("wg_sb", [128, 8 * 512], F32R)
    bias_sb = nc.alloc_sbuf_tensor("bias_sb", [1, 512], F32R)
    i16_sb = nc.alloc_sbuf_tensor("i16_sb", [128, 128], BF16)
    ones_sb = nc.alloc_sbuf_tensor("ones_sb", [1, 128], F32R)
    xstage = nc.alloc_sbuf_tensor("xstage", [128, 8 * 128], F32R)
    ring = nc.alloc_sbuf_tensor("ring", [128, 512], BF16)
    recv = [nc.alloc_sbuf_tensor(f"recv{p}", [128, 128], BF16) for p in range(2)]
    src = [nc.alloc_sbuf_tensor(f"srcb{p}", [128, 32], BF16) for p in range(2)]
    G = [nc.alloc_sbuf_tensor(f"G{p}", [16, 512], BF16) for p in range(2)]
    TC = [nc.alloc_sbuf_tensor(f"TC{p}", [16, 128], BF16) for p in range(2)]
    hb = [nc.alloc_sbuf_tensor(f"hb{p}", [32, 128], BF16) for p in range(2)]
    C = nc.alloc_sbuf_tensor("C", [16, 128], BF16)
    M1 = nc.alloc_sbuf_tensor("M1", [16, 128], BF16)
    M2 = nc.alloc_sbuf_tensor("M2", [16, 128], BF16)
    M3 = nc.alloc_sbuf_tensor("M3", [16, 128], BF16)
    M4 = nc.alloc_sbuf_tensor("M4", [16, 128], BF16)
    T1 = nc.alloc_sbuf_tensor("T1", [16, 128], BF16)
    T2 = nc.alloc_sbuf_tensor("T2", [16, 128], BF16)

    z_ps = [nc.alloc_psum_tensor(f"z_ps{p}", [16, 512], F32) for p in range(2)]
    g_ps = nc.alloc_psum_tensor("g_ps", [128, 512], F32)
    scr_ps = nc.alloc_psum_tensor("scr_ps", [16, 512], F32)

    pe, act, dve, gp, sp = nc.tensor, nc.scalar, nc.vector, nc.gpsimd, nc.sync

    with (
        nc.semaphore("init_sem") as init_sem,
        nc.semaphore("prep_sem") as prep_sem,
        nc.semaphore("lsem") as lsem,
        nc.semaphore("rsem") as rsem,
        nc.semaphore("pe_z") as pe_z,
        nc.semaphore("pe_gemm") as pe_gemm,
        nc.semaphore("act1") as act1,
        nc.semaphore("act2") as act2,
        nc.semaphore("act_zx") as act_zx,
        nc.semaphore("dve_c") as dve_c,
        nc.semaphore("dve_h") as dve_h,
        nc.semaphore("dve_cp") as dve_cp,
        nc.semaphore("x_sem") as x_sem,
        nc.semaphore("out_dma") as out_dma,
    ):
        n_init = 0
        for d in range(8):
            sp.dma_start(out=wrec_sb[:, 512 * d:512 * d + 512],
                         in_=wrec[128 * d:128 * d + 128, :]).then_inc(init_sem, 16)
            sp.dma_start(out=wg_sb[:, 512 * d:512 * d + 512],
                         in_=wg[128 * d:128 * d + 128, :]).then_inc(init_sem, 16)
            n_init += 2
        for dst, srct in ((bias_sb, biasd), (i16_sb, i16d), (ones_sb, onesd)):
            sp.dma_start(out=dst[:], in_=srct[:]).then_inc(init_sem, 16)
            n_init += 1
        sp.wait_ge(init_sem, 16 * n_init)
        # dummy sigmoid pins the ACT func table to sigmoid_and_others (which
        # also holds tanh+copy) so the loop body never reloads tables
        act.wait_ge(init_sem, 16 * n_init)
        act.activation(T1[0:1, 0:1], i16_sb[0:1, 0:1], AF.Sigmoid)
        dve.memset(recv[0][:], 0.0)
        dve.memset(C[:], 0.0)
        dve.memset(hb[0][:], 0.0)
        dve.memset(hb[1][:], 0.0)
        dve.memset(M1[0:1, 0:1], 0.0).then_inc(dve_cp, 2)
        dve.memset(M2[0:1, 0:1], 0.0).then_inc(dve_c, 2)
        dve.memset(M4[0:1, 0:1], 0.0).then_inc(out_dma, 32)
        for k in range(2):
            gp.remote_sem_update_broadcast(
                remote_sem=rsem, local_sem=lsem,
                rdests=[(0, kk) for kk in range(8)],
            ).then_inc(prep_sem)
            gp.wait_ge(prep_sem, k + 1)
            gp.trigger_dma(1)
        nc.all_engine_barrier()

        xcol = sp.alloc_register("xcol"); sp.reg_mov(xcol, 0)
        orow = sp.alloc_register("orow"); sp.reg_mov(orow, 0)
        r_sp_dveh = sp.alloc_register("r_sp_dveh"); sp.reg_mov(r_sp_dveh, 0)
        r_sp_gemm = sp.alloc_register("r_sp_gemm"); sp.reg_mov(r_sp_gemm, 0)

        def x_block_dma():
            xoff = sp.snap(xcol)
            for k in range(8):
                sp.dma_start(
                    out=xstage[:, 128 * k:128 * k + 128],
                    in_=xt[128 * k:128 * k + 128, bass.ds(xoff, 128)],
                ).then_inc(x_sem, 16)
            sp.reg_add(xcol, xcol, 128)

        x_block_dma()

        pe.wait_ge(x_sem, 128)
        for k in range(8):
            pe.matmul(g_ps[:], xstage[:, 128 * k:128 * k + 128],
                      wg_sb[:, 512 * k:512 * k + 512],
                      start=(k == 0), stop=False, skip_group_check=True)
        pe.matmul(g_ps[:], ones_sb[0:1, :],
                  bias_sb[0:1, :],
                  start=False, stop=True, skip_group_check=True).then_inc(pe_gemm)

        act.wait_ge(pe_gemm, 1)
        act.activation(ring[:], g_ps[:], AF.Copy).then_inc(act_zx)

        r_pe_rsem = pe.alloc_register("r_pe_rsem"); pe.reg_mov(r_pe_rsem, 32)
        r_pe_x = pe.alloc_register("r_pe_x"); pe.reg_mov(r_pe_x, 128)
        r_pe_zx = pe.alloc_register("r_pe_zx"); pe.reg_mov(r_pe_zx, 0)
        r_a_pez = act.alloc_register("r_a_pez"); act.reg_mov(r_a_pez, 0)
        r_a_war = act.alloc_register("r_a_war"); act.reg_mov(r_a_war, 0)
        r_a_dvec = act.alloc_register("r_a_dvec"); act.reg_mov(r_a_dvec, 2)
        r_a_gemm = act.alloc_register("r_a_gemm"); act.reg_mov(r_a_gemm, 1)
        r_v_act1 = dve.alloc_register("r_v_act1"); dve.reg_mov(r_v_act1, 0)
        r_v_act2 = dve.alloc_register("r_v_act2"); dve.reg_mov(r_v_act2, 0)
        r_v_odma = dve.alloc_register("r_v_odma"); dve.reg_mov(r_v_odma, 0)
        r_g_prep = gp.alloc_register("r_g_prep"); gp.reg_mov(r_g_prep, 2)
        r_g_cp = gp.alloc_register("r_g_cp"); gp.reg_mov(r_g_cp, 2)

        NDUM = 6
        with pe.Fori(0, ITERS):
            pe.reg_add(r_pe_zx, r_pe_zx, 1)
            for j in range(8):
                par = (j + 1) % 2
                rpar = j % 2
                # pre-wait zone: identity (zx), gemm share, and ramp-warmer
                # dummies all run while waiting for the h broadcast, keeping
                # the PE p-state ramped for the critical recurrence burst
                if j == 0:
                    pe.wait_ge(act_zx, r_pe_zx)
                pe.matmul(z_ps[par][:], i16_sb[:, 16 * j:16 * j + 16],
                          ring[:],
                          start=True, stop=False, skip_group_check=True)
                if j >= 1:
                    if j == 1:
                        pe.reg_add(r_pe_x, r_pe_x, 128)
                        pe.wait_ge(x_sem, r_pe_x)
                    k = j - 1
                    pe.matmul(g_ps[:], xstage[:, 128 * k:128 * k + 128],
                              wg_sb[:, 512 * k:512 * k + 512],
                              start=(k == 0), stop=False, skip_group_check=True)
                if j == 7:
                    pe.matmul(g_ps[:], xstage[:, 128 * 7:128 * 7 + 128],
                              wg_sb[:, 512 * 7:512 * 7 + 512],
                              start=False, stop=False, skip_group_check=True)
                    pe.matmul(g_ps[:], ones_sb[0:1, :],
                              bias_sb[0:1, :],
                              start=False, stop=True,
                              skip_group_check=True).then_inc(pe_gemm)
                for _ in range(NDUM):
                    pe.matmul(scr_ps[:], i16_sb[:, 0:16],
                              wrec_sb[:, 0:512],
                              start=True, stop=True, skip_group_check=True)
                pe.wait_ge(rsem, r_pe_rsem)
                pe.reg_add(r_pe_rsem, r_pe_rsem, 16)
                for d in range(8):
                    inst = pe.matmul(z_ps[par][:],
                                     recv[rpar][:, 16 * d:16 * d + 16],
                                     wrec_sb[:, 512 * d:512 * d + 512],
                                     start=False, stop=(d == 7),
                                     skip_group_check=True)
                inst.then_inc(pe_z)

        with act.Fori(0, ITERS):
            for j in range(8):
                par = (j + 1) % 2
                act.reg_add(r_a_pez, r_a_pez, 1)
                act.wait_ge(pe_z, r_a_pez)
                act.reg_add(r_a_war, r_a_war, 1)
                act.wait_ge(dve_c, r_a_war)
                act.activation(G[par][:, 0:128], z_ps[par][:, 0:128], AF.Tanh)
                act.activation(G[par][:, 128:384], z_ps[par][:, 128:384],
                               AF.Sigmoid).then_inc(act1)
                act.activation(G[par][:, 384:512], z_ps[par][:, 384:512],
                               AF.Sigmoid)
                act.reg_add(r_a_dvec, r_a_dvec, 1)
                act.wait_ge(dve_c, r_a_dvec)
                act.activation(TC[par][:], C[:], AF.Tanh).then_inc(act2)
            act.reg_add(r_a_gemm, r_a_gemm, 1)
            act.wait_ge(pe_gemm, r_a_gemm)
            act.activation(ring[:], g_ps[:], AF.Copy).then_inc(act_zx)

        with dve.Fori(0, ITERS):
            for j in range(8):
                par = (j + 1) % 2
                dve.reg_add(r_v_act1, r_v_act1, 1)
                dve.wait_ge(act1, r_v_act1)
                Gp = G[par]
                dve.tensor_mul(M1[:], Gp[:, 0:128], Gp[:, 128:256])
                dve.tensor_mul(M2[:, 0:64], Gp[:, 0:64], Gp[:, 192:256])
                dve.tensor_mul(M2[:, 64:128], Gp[:, 64:128], Gp[:, 128:192])
                dve.tensor_mul(M3[:], Gp[:, 256:384], C[:])
                dve.tensor_mul(M4[:, 0:64], Gp[:, 256:320], C[:, 64:128])
                dve.tensor_mul(M4[:, 64:128], Gp[:, 320:384], C[:, 0:64])
                dve.drain()
                dve.tensor_add(T1[:], M1[:], M3[:])
                dve.tensor_add(T2[:], M2[:], M4[:])
                dve.drain()
                dve.tensor_sub(C[:, 0:64], T1[:, 0:64], T1[:, 64:128])
                dve.tensor_add(C[:, 64:128], T2[:, 0:64],
                               T2[:, 64:128]).then_inc(dve_c)
                dve.reg_add(r_v_act2, r_v_act2, 1)
                dve.wait_ge(act2, r_v_act2)
                dve.tensor_mul(M1[:], Gp[:, 384:512], TC[par][:])
                dve.tensor_mul(M2[:, 0:64], Gp[:, 384:448], TC[par][:, 64:128])
                dve.tensor_mul(M2[:, 64:128], Gp[:, 448:512], TC[par][:, 0:64])
                dve.reg_add(r_v_odma, r_v_odma, 16)
                dve.wait_ge(out_dma, r_v_odma)
                dve.drain()
                dve.tensor_sub(hb[par][0:16, 0:64], M1[:, 0:64], M1[:, 64:128])
                dve.tensor_add(hb[par][0:16, 64:128], M2[:, 0:64],
                               M2[:, 64:128]).then_inc(dve_h)
                dve.drain()
                for kk in range(4):
                    inst = dve.transpose(
                        src[par][32 * kk:32 * kk + 32, 0:32],
                        hb[par][0:32, 32 * kk:32 * kk + 32])
                inst.then_inc(dve_cp)

        with sp.Fori(0, ITERS):
            sp.reg_add(r_sp_gemm, r_sp_gemm, 1)
            sp.wait_ge(pe_gemm, r_sp_gemm)
            x_block_dma()
            for j in range(8):
                par = (j + 1) % 2
                sp.reg_add(r_sp_dveh, r_sp_dveh, 1)
                sp.wait_ge(dve_h, r_sp_dveh)
                sp.dma_start(out=hout[bass.ds(sp.snap(orow), 16), :],
                             in_=hb[par][0:16, :]).then_inc(out_dma, 16)
                sp.reg_add(orow, orow, 16)

        pid = gp.partition_id()
        for c in gp.Switch(pid, 8):
            gp.remote_dma_broadcast(
                recv[1][:, 16 * c:16 * c + 16], src[1][:, 0:16],
                remote_sem=rsem, local_sem=lsem,
                rdests=[(0, k) for k in range(8)],
            ).then_inc(prep_sem)
            with gp.Fori(0, T // 2):
                for tp in (1, 0):
                    gp.reg_add(r_g_prep, r_g_prep, 1)
                    gp.wait_ge(prep_sem, r_g_prep)
                    gp.reg_add(r_g_cp, r_g_cp, 1)
                    gp.wait_ge(dve_cp, r_g_cp)
                    gp.trigger_dma(1)
                    gp.remote_dma_broadcast(
                        recv[1 - tp][:, 16 * c:16 * c + 16],
                        src[1 - tp][:, 0:16],
                        remote_sem=rsem, local_sem=lsem,
                        rdests=[(0, k) for k in range(8)],
                    ).then_inc(prep_sem)
            gp.reg_add(r_g_prep, r_g_prep, 1)
            gp.wait_ge(prep_sem, r_g_prep)
            gp.trigger_dma(1)

        gp.wait_ge(rsem, 32 + 16 * (T + 1))
        gp.wait_ge(lsem, 32 + 16 * (T + 1))
        sp.wait_ge(out_dma, 32 + 16 * T)
        nc.all_engine_barrier()

    nc.compile()
    return nc


def _round_tf32(a):
    """Zero the low 13 mantissa bits (tf32-style rounding for f32r)."""
    b = np.ascontiguousarray(a, np.float32).view(np.uint32)
    b = (b + 0x1000) & np.uint32(0xFFFFE000)
    return b.view(np.float32)


def _prep_inputs(x_real, x_imag, W_real, W_imag, U_real, U_imag,
                 b_real, b_imag, T):
    import ml_dtypes
    BF = ml_dtypes.bfloat16
    ITERS = T // 8
    NBLK = ITERS + 1
    xt = np.zeros((1024, NBLK * 128), np.float32)
    xr = np.asarray(x_real[:, :T], np.float32)
    xi = np.asarray(x_imag[:, :T], np.float32)
    xt[0:512, :T * 16] = xr.transpose(2, 1, 0).reshape(512, T * 16)
    xt[512:1024, :T * 16] = xi.transpose(2, 1, 0).reshape(512, T * 16)
    xt = _round_tf32(xt)

    i16 = np.eye(128, dtype=BF)
    ones1 = np.ones((1, 128), np.float32)
    Wr = np.asarray(W_real, np.float32); Wi = np.asarray(W_imag, np.float32)
    Ur = np.asarray(U_real, np.float32); Ui = np.asarray(U_imag, np.float32)
    br = np.asarray(b_real, np.float32); bi = np.asarray(b_imag, np.float32)

    in_maps = []
    for c in range(N_CORES):
        cols = np.zeros(512, np.int64)
        part = np.zeros(512, np.int64)
        for g in range(4):
            for pt in range(2):
                base = 128 * g + 64 * pt
                cols[base:base + 64] = 512 * g + 64 * c + np.arange(64)
                part[base:base + 64] = pt
        re = part == 0

        wgc = np.zeros((1024, 512), np.float32)
        wgc[0:512, re] = Wr[:, cols[re]]
        wgc[0:512, ~re] = Wi[:, cols[~re]]
        wgc[512:1024, re] = -Wi[:, cols[re]]
        wgc[512:1024, ~re] = Wr[:, cols[~re]]

        wrc = np.zeros((1024, 512), np.float32)
        for d in range(8):
            hf = 64 * d + np.arange(64)
            r0 = 128 * d
            wrc[r0:r0 + 64, re] = Ur[hf][:, cols[re]]
            wrc[r0:r0 + 64, ~re] = Ui[hf][:, cols[~re]]
            wrc[r0 + 64:r0 + 128, re] = -Ui[hf][:, cols[re]]
            wrc[r0 + 64:r0 + 128, ~re] = Ur[hf][:, cols[~re]]

        biasc = np.zeros((1, 512), np.float32)
        biasc[0, re] = br[cols[re]]
        biasc[0, ~re] = bi[cols[~re]]

        in_maps.append({
            "wrec": wrc.astype(BF), "wg": _round_tf32(wgc),
            "biasd": _round_tf32(biasc), "xt": xt,
            "i16d": i16, "onesd": ones1,
        })
    return in_maps


def kernel(x_real, x_imag, W_real, W_imag, U_real, U_imag, b_real, b_imag):
    T = int(np.asarray(x_real).shape[1])
    if T not in _cache:
        _cache[T] = _build(T)
    nc = _cache[T]
    in_maps = _prep_inputs(x_real, x_imag, W_real, W_imag, U_real, U_imag,
                           b_real, b_imag, T)
    res = bass_utils.run_bass_kernel_spmd(nc, in_maps,
                                          core_ids=list(range(N_CORES)))
    out = np.zeros((16, T, 512, 2), np.float32)
    for c in range(N_CORES):
        h = np.asarray(res.results[c]["hout"], np.float32).reshape(T, 16, 128)
        out[:, :, 64 * c:64 * c + 64, 0] = h[:, :, 0:64].transpose(1, 0, 2)
        out[:, :, 64 * c:64 * c + 64, 1] = h[:, :, 64:128].transpose(1, 0, 2)
    return out

